# revision 8
# baseline (speedup 1.0000x reference)
"""Interval-softmax diagonal bounds kernel for Trainium2 (8 NeuronCores).

Math (per row b, element i), identical to the reference after rewriting:
    e_u = exp(u), S_u = sum_j e_u[:, j]
    lower = e_l / (e_l - e_u + S_u)
    upper = e_u / (e_u - e_l + S_l)

Memory-bound problem: trade precision for bandwidth inside the 2e-2
tolerance (measured end-to-end max rel err ~0.7e-2):
  - inputs cast to fp16 on the host (|x| <= ~5.6 so abs err <= 2.8e-3,
    exp rel err <= 0.28%), packed as one [ROWS, l|u] dram tensor,
  - outputs leave the chip as bf16 (rel err <= 0.2%; fp16 would flush
    the ~1e-6 smallest outputs to subnormals), packed [ROWS, lo|up],
  halving HBM traffic to 8 MiB/core (~23.4 us at 358 GB/s per core).

Compute per 128-row block:
    ScalarE: exp(l)+rowsum, exp(u)+rowsum   (~2.0 us each)
    VectorE: 2x custom fused DVE op (8/8 ALU stages, ~2.3 us each):
        out = Src0 * recip1((Src0 - Src1) + C0)
    where recip1 is the bitcast-NOT seeded reciprocal with ONE
    Newton-Raphson step (minimax consts from RECIP_APPROX_FAST_CONSTS,
    max rel err 0.173%; the 2nd NR step is dropped to fit the final
    multiply into the 8-stage pipeline). Registered into
    concourse.dve_ops.OPS at import time (the documented extension
    point; shas computed in-process).

Schedule notes (from perfetto): HWDGE issues ride the serial Sync
sequencer, and an output-DMA's semaphore wait blocks every later issue
on that queue -- so all 4 input DMAs are emitted before any compute
(io pool holds 4 bufs) and outputs are emitted per block afterwards.
Block 0 splits l/u into separate transfers so exp(l) starts half a
transfer early; block 3 splits exp(l) and the DVE/store into column
halves so the drain tail after the last ACTIVATE is short.
"""

import os
import sys

import numpy as np

_REPO = "/opt/trn_rl_repo"
if _REPO not in sys.path:
    sys.path.insert(0, _REPO)

B, N = 4096, 2048
N_CORES = 8
ROWS = B // N_CORES  # 512 rows per core
P = 128
NBLK = ROWS // P     # 4 row-blocks per core
W = 2 * N            # packed l|u (and lower|upper) width
H = N // 2           # column half

_OP_NAME = "INTERVAL_SM_RECIP_MUL_ANT"
_SEED_C = -0.23549792   # Chebyshev seed scale (C1)
_NR_C = 2.0017324       # minimax 1-NR constant (C2)

_cache = {}


def _register_dve_op():
    """out = Src0 * recip1((Src0 - Src1) + C0); C0 = per-partition row sum.

    recip1: nx = bitnot(x); y0 = nx*C1; r = y0*(C2 - x*y0). 8 ALU
    stages exactly.
    """
    import concourse.dve_ops as dve_ops
    from concourse.dve_spec import (
        AluOp,
        Bin,
        C0,
        C1,
        C2,
        Spec,
        Src0,
        Src1,
        _has_src1,
        lower,
    )
    from concourse.dve_uop import DveOpSpec

    for o in dve_ops.OPS:
        if o.name == _OP_NAME:
            return o

    x = (Src0 - Src1) + C0
    nx = Bin(AluOp.BITWISE_NOT, x, x)
    y0 = nx * C1
    y1 = y0 * (C2 - x * y0)
    body = y1 * Src0

    def _ref(in0, in1, s0, s1, imm2):
        xx = (in0.astype(np.float32) - in1 + s0).astype(np.float32)
        nxx = (~xx.view(np.int32)).view(np.float32)
        yy0 = (nxx * np.float32(s1)).astype(np.float32)
        yy1 = (yy0 * (np.float32(imm2) - xx * yy0)).astype(np.float32)
        return (yy1 * in0).astype(np.float32)

    spec = Spec(body=body, reference=_ref)
    row = dve_ops._CUSTOM_DVE_ROW_BASE + len(dve_ops.OPS)
    assert row < 0x20, "custom-DVE opcode rows exhausted"
    shas = {}
    for ver in ("v3", "v4"):
        s = DveOpSpec(
            name=_OP_NAME,
            opcode=row,
            uops=lower(spec, ver=ver),
            rd1_en=_has_src1(spec),
        )
        shas[ver] = s.sha(ver)
    op = dve_ops.DveOp(_OP_NAME, spec, subdim=False, uops_sha=shas)
    dve_ops.OPS.append(op)
    dve_ops._SUB_OPCODE_FOR_NAME[_OP_NAME] = row
    dve_ops.CUSTOM_DVE_SPECS[_OP_NAME] = spec
    return op


def _build():
    import concourse.bacc as bacc
    import concourse.mybir as mybir
    import concourse.tile as tile

    op = _register_dve_op()
    f16 = mybir.dt.float16
    bf16 = mybir.dt.bfloat16
    f32 = mybir.dt.float32
    Exp = mybir.ActivationFunctionType.Exp
    Add = mybir.AluOpType.add
    nc = bacc.Bacc(
        "TRN2", target_bir_lowering=False, debug=False, num_devices=N_CORES
    )

    xu_d = nc.dram_tensor("xu", [ROWS, W], f16, kind="ExternalInput")
    out_d = nc.dram_tensor("out", [ROWS, W], bf16, kind="ExternalOutput")

    with tile.TileContext(nc) as tc:
        with (
            tc.tile_pool(name="io", bufs=3) as io,
            tc.tile_pool(name="eb", bufs=3) as eb,
            tc.tile_pool(name="ob", bufs=3) as ob,
            tc.tile_pool(name="stats", bufs=8) as st,
        ):
            # Phase 1: all input DMAs up front (io bufs cover all 4
            # blocks) so no output-DMA wait ever stalls an input issue.
            # Block 0 streams in as quarters, u first, so the first
            # lower-side DVE op can start as early as possible.
            xus = []
            for b in range(NBLK):
                rows = slice(b * P, (b + 1) * P)
                xu = io.tile([P, W], f16, tag="xu")
                if b == 0:
                    # first two quarters ride the (otherwise idle) Scalar
                    # HWDGE ring, which clears its preamble ~1.5us before
                    # Sync does -- pulls the whole exp/DVE chain earlier
                    nc.scalar.dma_start(out=xu[:, N : N + H], in_=xu_d[rows, N : N + H])
                    nc.scalar.dma_start(out=xu[:, N + H :], in_=xu_d[rows, N + H :])
                    nc.sync.dma_start(out=xu[:, 0:H], in_=xu_d[rows, 0:H])
                    nc.sync.dma_start(out=xu[:, H:N], in_=xu_d[rows, H:N])
                elif b in (1, 2):
                    # halves so each block's exp_l isn't starved
                    nc.sync.dma_start(out=xu[:, :N], in_=xu_d[rows, :N])
                    nc.sync.dma_start(out=xu[:, N:], in_=xu_d[rows, N:])
                else:
                    nc.sync.dma_start(out=xu, in_=xu_d[rows, :])
                xus.append(xu)

            # Phase 2: per-block compute + store.
            for b in range(NBLK):
                rows = slice(b * P, (b + 1) * P)
                xu = xus[b]
                e = eb.tile([P, W], f32, tag="e")
                s = st.tile([P, 6], f32, tag="s")
                o = ob.tile([P, W], bf16, tag="o")

                if b == 0:
                    # cols: s0=S_l_h0, s1=S_l_h1, s2=S_u_h0, s3=S_u_h1,
                    #       s4=S_u, s5=S_l
                    nc.scalar.activation(
                        e[:, N : N + H], xu[:, N : N + H], Exp,
                        accum_out=s[:, 2:3],
                    )
                    nc.scalar.activation(
                        e[:, N + H :], xu[:, N + H :], Exp, accum_out=s[:, 3:4]
                    )
                    nc.scalar.activation(
                        e[:, 0:H], xu[:, 0:H], Exp, accum_out=s[:, 0:1]
                    )
                    nc.scalar.activation(
                        e[:, H:N], xu[:, H:N], Exp, accum_out=s[:, 1:2]
                    )
                    nc.vector.tensor_scalar(
                        s[:, 4:5], s[:, 2:3], s[:, 3:4], None, op0=Add
                    )
                    nc.vector._custom_dve(
                        op, out=o[:, 0:H], in0=e[:, 0:H], in1=e[:, N : N + H],
                        s0=s[:, 4:5], s1=_SEED_C, imm2=_NR_C,
                    )
                    nc.vector._custom_dve(
                        op, out=o[:, H:N], in0=e[:, H:N], in1=e[:, N + H :],
                        s0=s[:, 4:5], s1=_SEED_C, imm2=_NR_C,
                    )
                    nc.sync.dma_start(out=out_d[rows, :N], in_=o[:, :N])
                    nc.vector.tensor_scalar(
                        s[:, 5:6], s[:, 0:1], s[:, 1:2], None, op0=Add
                    )
                    nc.vector._custom_dve(
                        op, out=o[:, N:], in0=e[:, N:], in1=e[:, :N],
                        s0=s[:, 5:6], s1=_SEED_C, imm2=_NR_C,
                    )
                    nc.sync.dma_start(out=out_d[rows, N:], in_=o[:, N:])
                elif b < NBLK - 1:
                    # cols: s[:,0]=S_l, s[:,1]=S_u
                    nc.scalar.activation(
                        e[:, :N], xu[:, :N], Exp, accum_out=s[:, 0:1]
                    )
                    nc.scalar.activation(
                        e[:, N:], xu[:, N:], Exp, accum_out=s[:, 1:2]
                    )
                    # lower = e_l * recip1(e_l - e_u + S_u)
                    nc.vector._custom_dve(
                        op, out=o[:, :N], in0=e[:, :N], in1=e[:, N:],
                        s0=s[:, 1:2], s1=_SEED_C, imm2=_NR_C,
                    )
                    # upper = e_u * recip1(e_u - e_l + S_l)
                    nc.vector._custom_dve(
                        op, out=o[:, N:], in0=e[:, N:], in1=e[:, :N],
                        s0=s[:, 0:1], s1=_SEED_C, imm2=_NR_C,
                    )
                    nc.sync.dma_start(out=out_d[rows, :], in_=o)
                else:
                    # Last block: exp(u) first, then exp(l) in column
                    # halves; the lower-side DVE ops chase the halves,
                    # and upper (gated on full S_l) runs in halves with
                    # quarter stores so the post-ACT tail is short.
                    # cols: s[:,0]=S_l_h0, s[:,1]=S_l_h1, s[:,2]=S_u,
                    #       s[:,3]=S_l
                    nc.scalar.activation(
                        e[:, N:], xu[:, N:], Exp, accum_out=s[:, 2:3]
                    )
                    nc.scalar.activation(
                        e[:, 0:H], xu[:, 0:H], Exp, accum_out=s[:, 0:1]
                    )
                    nc.scalar.activation(
                        e[:, H:N], xu[:, H:N], Exp, accum_out=s[:, 1:2]
                    )
                    nc.vector._custom_dve(
                        op, out=o[:, 0:H], in0=e[:, 0:H], in1=e[:, N : N + H],
                        s0=s[:, 2:3], s1=_SEED_C, imm2=_NR_C,
                    )
                    nc.sync.dma_start(out=out_d[rows, 0:H], in_=o[:, 0:H])
                    nc.vector._custom_dve(
                        op, out=o[:, H:N], in0=e[:, H:N], in1=e[:, N + H :],
                        s0=s[:, 2:3], s1=_SEED_C, imm2=_NR_C,
                    )
                    nc.sync.dma_start(out=out_d[rows, H:N], in_=o[:, H:N])
                    nc.vector.tensor_scalar(
                        s[:, 3:4], s[:, 0:1], s[:, 1:2], None, op0=Add
                    )
                    nc.vector._custom_dve(
                        op, out=o[:, N : N + H], in0=e[:, N : N + H],
                        in1=e[:, 0:H], s0=s[:, 3:4], s1=_SEED_C, imm2=_NR_C,
                    )
                    nc.sync.dma_start(
                        out=out_d[rows, N : N + H], in_=o[:, N : N + H]
                    )
                    nc.vector._custom_dve(
                        op, out=o[:, N + H :], in0=e[:, N + H :],
                        in1=e[:, H:N], s0=s[:, 3:4], s1=_SEED_C, imm2=_NR_C,
                    )
                    # last store in two pieces: the final (tail-critical)
                    # transfer is only 128 KiB
                    Q = H // 2
                    nc.sync.dma_start(
                        out=out_d[rows, N + H : N + H + Q],
                        in_=o[:, N + H : N + H + Q],
                    )
                    nc.sync.dma_start(
                        out=out_d[rows, N + H + Q :], in_=o[:, N + H + Q :]
                    )

    nc.compile()
    return nc


def _get_nc():
    if "nc" not in _cache:
        _cache["nc"] = _build()
    return _cache["nc"]


def kernel(l: np.ndarray, u: np.ndarray):
    from concourse import bass_utils

    assert l.shape == (B, N) and u.shape == (B, N)
    xu = np.empty((B, W), dtype=np.float16)
    xu[:, :N] = l
    xu[:, N:] = u

    nc = _get_nc()
    in_maps = [{"xu": xu[i * ROWS : (i + 1) * ROWS]} for i in range(N_CORES)]
    trace = bool(int(os.environ.get("KERNEL_TRACE", "0")))
    res = bass_utils.run_bass_kernel_spmd(
        nc,
        in_maps,
        core_ids=list(range(N_CORES)),
        trace=trace,
        trace_cores=[0] if trace else None,
    )
    _cache["last_run"] = res
    full = np.concatenate(
        [np.asarray(r["out"]) for r in res.results], axis=0
    ).astype(np.float32)
    return full[:, :N], full[:, N:]


# revision 10
# speedup vs baseline: 1.0007x; 1.0007x over previous
"""Interval-softmax diagonal bounds kernel for Trainium2 (8 NeuronCores).

Math (per row b, element i), identical to the reference after rewriting:
    e_u = exp(u), S_u = sum_j e_u[:, j]
    lower = e_l / (e_l - e_u + S_u)
    upper = e_u / (e_u - e_l + S_l)

Memory-bound problem: trade precision for bandwidth inside the 2e-2
tolerance (measured end-to-end max rel err ~0.7e-2):
  - inputs cast to fp16 on the host (|x| <= ~5.6 so abs err <= 2.8e-3,
    exp rel err <= 0.28%), packed as one [ROWS, l|u] dram tensor,
  - outputs leave the chip as bf16 (rel err <= 0.2%; fp16 would flush
    the ~1e-6 smallest outputs to subnormals), packed [ROWS, lo|up],
  halving HBM traffic to 8 MiB/core (~23.4 us at 358 GB/s per core).

Compute per 128-row block:
    ScalarE: exp(l)+rowsum, exp(u)+rowsum   (~2.0 us each)
    VectorE: 2x custom fused DVE op (8/8 ALU stages, ~2.3 us each):
        out = Src0 * recip1((Src0 - Src1) + C0)
    where recip1 is the bitcast-NOT seeded reciprocal with ONE
    Newton-Raphson step (minimax consts from RECIP_APPROX_FAST_CONSTS,
    max rel err 0.173%; the 2nd NR step is dropped to fit the final
    multiply into the 8-stage pipeline). Registered into
    concourse.dve_ops.OPS at import time (the documented extension
    point; shas computed in-process).

Schedule notes (from perfetto): HWDGE issues ride the serial Sync
sequencer, and an output-DMA's semaphore wait blocks every later issue
on that queue -- so all 4 input DMAs are emitted before any compute
(io pool holds 4 bufs) and outputs are emitted per block afterwards.
Block 0 splits l/u into separate transfers so exp(l) starts half a
transfer early; block 3 splits exp(l) and the DVE/store into column
halves so the drain tail after the last ACTIVATE is short.
"""

import os
import sys

import numpy as np

_REPO = "/opt/trn_rl_repo"
if _REPO not in sys.path:
    sys.path.insert(0, _REPO)

B, N = 4096, 2048
N_CORES = 8
ROWS = B // N_CORES  # 512 rows per core
P = 128
NBLK = ROWS // P     # 4 row-blocks per core
W = 2 * N            # packed l|u (and lower|upper) width
H = N // 2           # column half

_OP_NAME = "INTERVAL_SM_RECIP_MUL_ANT"
_SEED_C = -0.23549792   # Chebyshev seed scale (C1)
_NR_C = 2.0017324       # minimax 1-NR constant (C2)

_cache = {}


def _register_dve_op():
    """out = Src0 * recip1((Src0 - Src1) + C0); C0 = per-partition row sum.

    recip1: nx = bitnot(x); y0 = nx*C1; r = y0*(C2 - x*y0). 8 ALU
    stages exactly.
    """
    import concourse.dve_ops as dve_ops
    from concourse.dve_spec import (
        AluOp,
        Bin,
        C0,
        C1,
        C2,
        Spec,
        Src0,
        Src1,
        _has_src1,
        lower,
    )
    from concourse.dve_uop import DveOpSpec

    for o in dve_ops.OPS:
        if o.name == _OP_NAME:
            return o

    x = (Src0 - Src1) + C0
    nx = Bin(AluOp.BITWISE_NOT, x, x)
    y0 = nx * C1
    y1 = y0 * (C2 - x * y0)
    body = y1 * Src0

    def _ref(in0, in1, s0, s1, imm2):
        xx = (in0.astype(np.float32) - in1 + s0).astype(np.float32)
        nxx = (~xx.view(np.int32)).view(np.float32)
        yy0 = (nxx * np.float32(s1)).astype(np.float32)
        yy1 = (yy0 * (np.float32(imm2) - xx * yy0)).astype(np.float32)
        return (yy1 * in0).astype(np.float32)

    spec = Spec(body=body, reference=_ref)
    row = dve_ops._CUSTOM_DVE_ROW_BASE + len(dve_ops.OPS)
    assert row < 0x20, "custom-DVE opcode rows exhausted"
    shas = {}
    for ver in ("v3", "v4"):
        s = DveOpSpec(
            name=_OP_NAME,
            opcode=row,
            uops=lower(spec, ver=ver),
            rd1_en=_has_src1(spec),
        )
        shas[ver] = s.sha(ver)
    op = dve_ops.DveOp(_OP_NAME, spec, subdim=False, uops_sha=shas)
    dve_ops.OPS.append(op)
    dve_ops._SUB_OPCODE_FOR_NAME[_OP_NAME] = row
    dve_ops.CUSTOM_DVE_SPECS[_OP_NAME] = spec
    return op


def _build():
    import concourse.bacc as bacc
    import concourse.mybir as mybir
    import concourse.tile as tile

    op = _register_dve_op()
    f16 = mybir.dt.float16
    bf16 = mybir.dt.bfloat16
    f32 = mybir.dt.float32
    Exp = mybir.ActivationFunctionType.Exp
    Add = mybir.AluOpType.add
    nc = bacc.Bacc(
        "TRN2", target_bir_lowering=False, debug=False, num_devices=N_CORES
    )

    xu_d = nc.dram_tensor("xu", [ROWS, W], f16, kind="ExternalInput")
    out_d = nc.dram_tensor("out", [ROWS, W], bf16, kind="ExternalOutput")

    with tile.TileContext(nc) as tc:
        with (
            tc.tile_pool(name="io", bufs=3) as io,
            tc.tile_pool(name="eb", bufs=3) as eb,
            tc.tile_pool(name="ob", bufs=3) as ob,
            tc.tile_pool(name="stats", bufs=8) as st,
        ):
            # Phase 1: all input DMAs up front (io bufs cover all 4
            # blocks) so no output-DMA wait ever stalls an input issue.
            # Block 0 streams in as quarters, u first, so the first
            # lower-side DVE op can start as early as possible.
            xus = []
            for b in range(NBLK):
                rows = slice(b * P, (b + 1) * P)
                xu = io.tile([P, W], f16, tag="xu")
                if b == 0:
                    nc.sync.dma_start(out=xu[:, N : N + H], in_=xu_d[rows, N : N + H])
                    nc.sync.dma_start(out=xu[:, N + H :], in_=xu_d[rows, N + H :])
                    nc.sync.dma_start(out=xu[:, 0:H], in_=xu_d[rows, 0:H])
                    nc.sync.dma_start(out=xu[:, H:N], in_=xu_d[rows, H:N])
                elif b in (1, 2):
                    # halves so each block's exp_l isn't starved
                    nc.sync.dma_start(out=xu[:, :N], in_=xu_d[rows, :N])
                    nc.sync.dma_start(out=xu[:, N:], in_=xu_d[rows, N:])
                else:
                    nc.sync.dma_start(out=xu, in_=xu_d[rows, :])
                xus.append(xu)

            # Phase 2: per-block compute + store.
            for b in range(NBLK):
                rows = slice(b * P, (b + 1) * P)
                xu = xus[b]
                e = eb.tile([P, W], f16, tag="e")
                s = st.tile([P, 6], f32, tag="s")
                o = ob.tile([P, W], bf16, tag="o")

                if b == 0:
                    # cols: s0=S_l_h0, s1=S_l_h1, s2=S_u_h0, s3=S_u_h1,
                    #       s4=S_u, s5=S_l
                    nc.scalar.activation(
                        e[:, N : N + H], xu[:, N : N + H], Exp,
                        accum_out=s[:, 2:3],
                    )
                    nc.scalar.activation(
                        e[:, N + H :], xu[:, N + H :], Exp, accum_out=s[:, 3:4]
                    )
                    nc.scalar.activation(
                        e[:, 0:H], xu[:, 0:H], Exp, accum_out=s[:, 0:1]
                    )
                    nc.scalar.activation(
                        e[:, H:N], xu[:, H:N], Exp, accum_out=s[:, 1:2]
                    )
                    nc.vector.tensor_scalar(
                        s[:, 4:5], s[:, 2:3], s[:, 3:4], None, op0=Add
                    )
                    nc.vector._custom_dve(
                        op, out=o[:, 0:H], in0=e[:, 0:H], in1=e[:, N : N + H],
                        s0=s[:, 4:5], s1=_SEED_C, imm2=_NR_C,
                    )
                    nc.vector._custom_dve(
                        op, out=o[:, H:N], in0=e[:, H:N], in1=e[:, N + H :],
                        s0=s[:, 4:5], s1=_SEED_C, imm2=_NR_C,
                    )
                    nc.sync.dma_start(out=out_d[rows, :N], in_=o[:, :N])
                    nc.vector.tensor_scalar(
                        s[:, 5:6], s[:, 0:1], s[:, 1:2], None, op0=Add
                    )
                    nc.vector._custom_dve(
                        op, out=o[:, N:], in0=e[:, N:], in1=e[:, :N],
                        s0=s[:, 5:6], s1=_SEED_C, imm2=_NR_C,
                    )
                    nc.sync.dma_start(out=out_d[rows, N:], in_=o[:, N:])
                elif b < NBLK - 1:
                    # cols: s[:,0]=S_l, s[:,1]=S_u
                    nc.scalar.activation(
                        e[:, :N], xu[:, :N], Exp, accum_out=s[:, 0:1]
                    )
                    nc.scalar.activation(
                        e[:, N:], xu[:, N:], Exp, accum_out=s[:, 1:2]
                    )
                    # lower = e_l * recip1(e_l - e_u + S_u)
                    nc.vector._custom_dve(
                        op, out=o[:, :N], in0=e[:, :N], in1=e[:, N:],
                        s0=s[:, 1:2], s1=_SEED_C, imm2=_NR_C,
                    )
                    # upper = e_u * recip1(e_u - e_l + S_l)
                    nc.vector._custom_dve(
                        op, out=o[:, N:], in0=e[:, N:], in1=e[:, :N],
                        s0=s[:, 0:1], s1=_SEED_C, imm2=_NR_C,
                    )
                    nc.sync.dma_start(out=out_d[rows, :], in_=o)
                else:
                    # Last block: exp(u) first, then exp(l) in column
                    # halves; the lower-side DVE ops chase the halves,
                    # and upper (gated on full S_l) runs in halves with
                    # quarter stores so the post-ACT tail is short.
                    # cols: s[:,0]=S_l_h0, s[:,1]=S_l_h1, s[:,2]=S_u,
                    #       s[:,3]=S_l
                    nc.scalar.activation(
                        e[:, N:], xu[:, N:], Exp, accum_out=s[:, 2:3]
                    )
                    nc.scalar.activation(
                        e[:, 0:H], xu[:, 0:H], Exp, accum_out=s[:, 0:1]
                    )
                    nc.scalar.activation(
                        e[:, H:N], xu[:, H:N], Exp, accum_out=s[:, 1:2]
                    )
                    nc.vector._custom_dve(
                        op, out=o[:, 0:H], in0=e[:, 0:H], in1=e[:, N : N + H],
                        s0=s[:, 2:3], s1=_SEED_C, imm2=_NR_C,
                    )
                    nc.sync.dma_start(out=out_d[rows, 0:H], in_=o[:, 0:H])
                    nc.vector._custom_dve(
                        op, out=o[:, H:N], in0=e[:, H:N], in1=e[:, N + H :],
                        s0=s[:, 2:3], s1=_SEED_C, imm2=_NR_C,
                    )
                    nc.sync.dma_start(out=out_d[rows, H:N], in_=o[:, H:N])
                    nc.vector.tensor_scalar(
                        s[:, 3:4], s[:, 0:1], s[:, 1:2], None, op0=Add
                    )
                    nc.vector._custom_dve(
                        op, out=o[:, N : N + H], in0=e[:, N : N + H],
                        in1=e[:, 0:H], s0=s[:, 3:4], s1=_SEED_C, imm2=_NR_C,
                    )
                    nc.sync.dma_start(
                        out=out_d[rows, N : N + H], in_=o[:, N : N + H]
                    )
                    nc.vector._custom_dve(
                        op, out=o[:, N + H :], in0=e[:, N + H :],
                        in1=e[:, H:N], s0=s[:, 3:4], s1=_SEED_C, imm2=_NR_C,
                    )
                    # last store in two pieces: the final (tail-critical)
                    # transfer is only 128 KiB
                    Q = H // 2
                    nc.sync.dma_start(
                        out=out_d[rows, N + H : N + H + Q],
                        in_=o[:, N + H : N + H + Q],
                    )
                    nc.sync.dma_start(
                        out=out_d[rows, N + H + Q :], in_=o[:, N + H + Q :]
                    )

    nc.compile()
    return nc


def _get_nc():
    if "nc" not in _cache:
        _cache["nc"] = _build()
    return _cache["nc"]


def kernel(l: np.ndarray, u: np.ndarray):
    from concourse import bass_utils

    assert l.shape == (B, N) and u.shape == (B, N)
    xu = np.empty((B, W), dtype=np.float16)
    xu[:, :N] = l
    xu[:, N:] = u

    nc = _get_nc()
    in_maps = [{"xu": xu[i * ROWS : (i + 1) * ROWS]} for i in range(N_CORES)]
    trace = bool(int(os.environ.get("KERNEL_TRACE", "0")))
    res = bass_utils.run_bass_kernel_spmd(
        nc,
        in_maps,
        core_ids=list(range(N_CORES)),
        trace=trace,
        trace_cores=[0] if trace else None,
    )
    _cache["last_run"] = res
    full = np.concatenate(
        [np.asarray(r["out"]) for r in res.results], axis=0
    ).astype(np.float32)
    return full[:, :N], full[:, N:]
